# revision 42
# baseline (speedup 1.0000x reference)
"""Trainium2 Bass kernel for BlockPrototypeMemory (sparse block attention).

Computation (reference):
  mem = MLP(mem_params)            # (P=64, NB=16, DB=128) rows through 128->512->512->512->128 MLP
  khat = block_ln(mem)             # LayerNorm per (p, m) row over DB
  qhat = block_ln(queries)         # LayerNorm per (token, m) block over DB
  logits[b,m,n,p] = qhat . khat / sqrt(DB)
  out = softmax_p(logits) @ khat

Key algebraic trick: khat rows are exactly zero-mean over DB (LayerNorm output), so
  logits = (1/(sigma_q sqrt(DB))) * (q_raw . khat)
i.e. q's mean subtraction is unnecessary; only the per-(token,block) scale
c = 1/sqrt((var+eps)*DB) must be applied to q before the matmul.

Sharding: data-parallel over B (8 batches -> 8 cores), MLP + mem replicated per core.
"""

import os
import sys

sys.path.insert(0, "/opt/trn_rl_repo")

import numpy as np
import ml_dtypes
from contextlib import ExitStack

from concourse import bass, mybir, tile, masks
from concourse.bass_utils import run_bass_kernel_spmd

AF = mybir.ActivationFunctionType
ALU = mybir.AluOpType
DT = mybir.dt

P, NB, D, DB, H = 64, 16, 2048, 128, 512
EPS = 1e-5
N_CORES = 8
N_TOKENS = 4096
CHUNK = 512          # tokens per macro-iteration
TPC = CHUNK // 128   # 128-token tiles per chunk

def emit_kernel(ctx: ExitStack, tc: "tile.TileContext", outs, ins, n_tokens=N_TOKENS,
                repeats=1):
    """Emit the per-core kernel. ins/outs are dicts of DRAM APs."""
    nc = tc.nc
    q_ext = ins["q"]          # [n_tokens, D] bf16   (token-major)
    mp_ext = ins["mp"]        # [DB, NB*P] bf16  feature-major, cols ordered (m, p)
    w1_ext = ins["w1"]        # [DB, H] bf16
    w2_ext = ins["w2"]        # [H, H] bf16
    w3_ext = ins["w3"]        # [H, H] bf16
    w4_ext = ins["w4"]        # [H, DB] bf16
    b123_ext = ins["b123"]    # [128, 12] f32 (b1|b2|b3 each reshaped (4,128).T)
    b4r_ext = ins["b4r"]      # [1, 128] bf16
    out_ext = outs["out"]     # [n_tokens, D] f32

    n_chunks = n_tokens // CHUNK
    NROW = NB * P            # 1024 rows through the MLP
    NRT = NROW // 128        # 8 row-tiles

    # ---------------- constant tiles ----------------
    const_pool = ctx.enter_context(tc.tile_pool(name="const", bufs=1))
    ident_bf = const_pool.tile([128, 128], DT.bfloat16, tag="identbf")
    ident_f32 = const_pool.tile([128, 128], DT.float32, tag="identf32")
    masks.make_identity(nc, ident_bf[:])
    masks.make_identity(nc, ident_f32[:])
    ones2_bf = const_pool.tile([128, 2], DT.bfloat16, tag="ones2")
    nc.gpsimd.memset(ones2_bf[:], 0.0)
    nc.gpsimd.memset(ones2_bf[0:64, 0:1], 1.0)
    nc.gpsimd.memset(ones2_bf[64:128, 1:2], 1.0)
    epsb = const_pool.tile([128, 1], DT.float32, tag="epsb")
    nc.gpsimd.memset(epsb[:], float(128.0 * EPS))
    # S_all[:, j*128:(j+1)*128] is the [16,128] stationary that replicates
    # rT rows (2j, 2j+1) onto partition halves (0:64, 64:128) via matmul
    s_all = const_pool.tile([16, NB // 2 * 128], DT.bfloat16, tag="sall")
    nc.sync.dma_start(s_all[:], ins["sall"][:, :])

    # persistent khat tiles
    khat_pool = ctx.enter_context(tc.tile_pool(name="khat", bufs=1))
    # k2t: transposed khat, feature-major per pair: cols [ro*128 : ro*128+64] = m=2ro, next 64 = m=2ro+1
    k2t = khat_pool.tile([128, NRT * 128], DT.bfloat16, tag="k2t")
    # krt_e/krt_o: row-major khat with the other pair-half zeroed, so the
    # out-projection can use full-K=128 operands (no partition-offset APs)
    krt_e = khat_pool.tile([128, NRT * 128], DT.bfloat16, tag="krte")
    krt_o = khat_pool.tile([128, NRT * 128], DT.bfloat16, tag="krto")
    nc.gpsimd.memset(krt_e[:], 0.0)
    nc.gpsimd.memset(krt_o[:], 0.0)

    # ---------------- MLP + khat setup (bf16 matmuls, fp32 LN) ----------------
    def emit_mlp(sctx):
        mw = sctx.enter_context(tc.tile_pool(name="mlpw", bufs=1))
        mact = sctx.enter_context(tc.tile_pool(name="mlpact", bufs=1))
        msc = sctx.enter_context(tc.tile_pool(name="mlpsc", bufs=2))

        w1_sb = mw.tile([128, H], DT.bfloat16, tag="w1")
        nc.sync.dma_start(w1_sb[:], w1_ext[:, :])
        w2_sb = mw.tile([128, 4 * H], DT.bfloat16, tag="w2")
        w3_sb = mw.tile([128, 4 * H], DT.bfloat16, tag="w3")
        for ki in range(4):
            nc.sync.dma_start(w2_sb[:, bass.ts(ki, H)], w2_ext[bass.ts(ki, 128), :])
            nc.sync.dma_start(w3_sb[:, bass.ts(ki, H)], w3_ext[bass.ts(ki, 128), :])
        w4_sb = mw.tile([128, 4 * 128], DT.bfloat16, tag="w4")
        for ki in range(4):
            nc.sync.dma_start(w4_sb[:, bass.ts(ki, 128)], w4_ext[bass.ts(ki, 128), :])
        b123_sb = mw.tile([128, 12], DT.float32, tag="b123")
        nc.sync.dma_start(b123_sb[:], b123_ext[:, :])
        b4r_sb = mw.tile([1, 128], DT.bfloat16, tag="b4r")
        nc.sync.dma_start(b4r_sb[:], b4r_ext[:, :])
        ones_row_bf = mw.tile([1, 128], DT.bfloat16, tag="onesrowbf")
        nc.gpsimd.memset(ones_row_bf[:], 1.0)

        # mem_params arrive host-transposed (feature-major) in bf16
        x_fm = mact.tile([128, NROW], DT.bfloat16, tag="xfm")
        nc.sync.dma_start(x_fm[:], mp_ext[:, :])

        # L1: h1[mo] = relu(W1[:,mo].T @ x + b1[mo])   feature-major [128, NROW] x4
        h1 = mact.tile([128, 4 * NROW], DT.bfloat16, tag="h1")
        for mo in range(4):
            for ch in range(NROW // 512):
                ps = lg_ps.tile([128, 512], DT.float32, tag="lg")
                nc.tensor.matmul(
                    ps[:], w1_sb[:, bass.ts(mo, 128)],
                    x_fm[:, bass.ts(ch, 512)], start=True, stop=True)
                nc.scalar.activation(
                    h1[:, mo * NROW + ch * 512:mo * NROW + ch * 512 + 512], ps[:],
                    AF.Relu, bias=b123_sb[:, mo:mo + 1])
        # L2 / L3 (L3 reuses h1's buffer: h1 is dead once L2 is done)
        hprev = h1
        for li, (w_sb, boff) in enumerate([(w2_sb, 4), (w3_sb, 8)]):
            hnext = mact.tile([128, 4 * NROW], DT.bfloat16, tag=["h2", "h1"][li])
            for mo in range(4):
                for ch in range(NROW // 512):
                    ps = lg_ps.tile([128, 512], DT.float32, tag="lg")
                    for ki in range(4):
                        nc.tensor.matmul(
                            ps[:],
                            w_sb[:, ki * H + mo * 128:ki * H + mo * 128 + 128],
                            hprev[:, ki * NROW + ch * 512:ki * NROW + ch * 512 + 512],
                            start=(ki == 0), stop=(ki == 3))
                    nc.scalar.activation(
                        hnext[:, mo * NROW + ch * 512:mo * NROW + ch * 512 + 512], ps[:],
                        AF.Relu, bias=b123_sb[:, boff + mo:boff + mo + 1])
            hprev = hnext

        # L4 (token-major out) + bias via rank-1 + block LN -> khat
        for ro in range(NRT):
            ps4 = dn_ps.tile([128, 128], DT.float32, tag="dn")
            for ki in range(4):
                nc.tensor.matmul(
                    ps4[:],
                    hprev[:, ki * NROW + ro * 128:ki * NROW + ro * 128 + 128],
                    w4_sb[:, bass.ts(ki, 128)],
                    start=(ki == 0), stop=False)
            nc.tensor.matmul(ps4[:], ones_row_bf[:], b4r_sb[:],
                             start=False, stop=True)
            # row LayerNorm stats (over 128 features)
            st = msc.tile([128, 6], DT.float32, tag="mst")
            nc.vector.bn_stats(st[:], ps4[:])
            mean = msc.tile([128, 1], DT.float32, tag="mmean")
            var = msc.tile([128, 1], DT.float32, tag="mvar")
            tmp = msc.tile([128, 1], DT.float32, tag="mtmp")
            nc.vector.tensor_add(mean[:], st[:, 1:2], st[:, 4:5])
            nc.vector.tensor_scalar_mul(mean[:], mean[:], 0.5)
            nc.vector.tensor_add(var[:], st[:, 2:3], st[:, 5:6])
            nc.vector.tensor_sub(tmp[:], st[:, 1:2], st[:, 4:5])
            nc.vector.tensor_mul(tmp[:], tmp[:], tmp[:])
            nc.vector.tensor_scalar(var[:], var[:], 1.0 / 128.0, None, ALU.mult)
            nc.vector.tensor_scalar_mul(tmp[:], tmp[:], 0.25)
            nc.vector.tensor_add(var[:], var[:], tmp[:])
            # sc = rsqrt(var+eps) = exp(-0.5*ln(var+eps)); nbias = -mean*sc
            sc = msc.tile([128, 1], DT.float32, tag="msc")
            nc.vector.tensor_scalar_add(sc[:], var[:], EPS)
            nc.scalar.activation(sc[:], sc[:], AF.Ln)
            nc.scalar.activation(sc[:], sc[:], AF.Exp, scale=-0.5)
            nbias = msc.tile([128, 1], DT.float32, tag="mnb")
            nc.vector.tensor_mul(nbias[:], mean[:], sc[:])
            nc.vector.tensor_scalar_mul(nbias[:], nbias[:], -1.0)
            ktm = msc.tile([128, 128], DT.float32, tag="ktm")
            nc.scalar.activation(ktm[:], ps4[:], AF.Identity, bias=nbias[:], scale=sc[:])
            # bf16 masked row-major copies + transposed copy
            nc.scalar.copy(krt_e[0:64, bass.ts(ro, 128)], ktm[0:64, :])
            nc.scalar.copy(krt_o[64:128, bass.ts(ro, 128)], ktm[64:128, :])
            ptk = dn_ps.tile([128, 128], DT.float32, tag="dn")
            nc.tensor.transpose(ptk[:], ktm[:], ident_f32[:])
            nc.scalar.copy(k2t[:, bass.ts(ro, 128)], ptk[:])

    # ---------------- attention over token chunks ----------------
    q_pool = ctx.enter_context(tc.tile_pool(name="qin", bufs=10))
    qs_pool = ctx.enter_context(tc.tile_pool(name="qs", bufs=8))
    qsT_pool = ctx.enter_context(tc.tile_pool(name="qsT", bufs=3))
    e_pool = ctx.enter_context(tc.tile_pool(name="eexp", bufs=10))
    e2_pool = ctx.enter_context(tc.tile_pool(name="ehat", bufs=10))
    o_pool = ctx.enter_context(tc.tile_pool(name="osb", bufs=4))
    stat_pool = ctx.enter_context(tc.tile_pool(name="stat", bufs=10))
    tp_ps = ctx.enter_context(tc.tile_pool(name="tp_ps", bufs=2, space="PSUM"))
    lg_ps = ctx.enter_context(tc.tile_pool(name="lg_ps", bufs=2, space="PSUM"))
    op_ps = ctx.enter_context(tc.tile_pool(name="op_ps", bufs=2, space="PSUM"))
    dn_ps = ctx.enter_context(tc.tile_pool(name="dn_ps", bufs=2, space="PSUM"))

    qscale_pool = int(os.environ.get("QSCALE_POOL", "13"))
    copy_dve = int(os.environ.get("COPY_DVE", "4"))
    evcopy_dve = int(os.environ.get("EVCOPY_DVE", "0"))

    def front(c0):
        """DMA + stats + scale + transpose for one chunk -> qsT tile."""
        q_t = []
        qs_t = []
        for t in range(TPC):
            qt = q_pool.tile([128, D], DT.bfloat16, tag="qt")
            nc.sync.dma_start(qt[:], q_ext[c0 + t * 128:c0 + t * 128 + 128, :])
            q_t.append(qt)
        for t in range(TPC):
            qt = q_t[t]
            st = stat_pool.tile([128, 16, 6], DT.float32, tag="st")
            # HW BNStats requires exactly 6 outputs/partition -> one instr per block
            for m in range(NB):
                nc.vector.bn_stats(st[:, m, :], qt[:, bass.ts(m, 128)])
            # V = 128*var = (st2+st5) + 32*(st1-st4)^2
            var = stat_pool.tile([128, 16], DT.float32, tag="var")
            tmp = stat_pool.tile([128, 16], DT.float32, tag="tmp")
            nc.vector.tensor_sub(tmp[:], st[:, :, 1], st[:, :, 4])
            nc.vector.scalar_tensor_tensor(tmp[:], tmp[:], 32.0, tmp[:], ALU.mult, ALU.mult)
            nc.vector.tensor_add(var[:], st[:, :, 2], st[:, :, 5])
            nc.vector.tensor_add(var[:], var[:], tmp[:])
            # c = rsqrt((var+eps)*DB) = exp(-0.5*ln(V + DB*eps))
            ct = stat_pool.tile([128, 16], DT.float32, tag="ct")
            nc.scalar.activation(ct[:], var[:], AF.Ln, bias=epsb[:])
            nc.scalar.activation(ct[:], ct[:], AF.Exp, scale=-0.5)
            qs = qs_pool.tile([128, D], DT.bfloat16, tag="qst")
            for m in range(NB):
                if (m * qscale_pool) % 16 < qscale_pool:
                    nc.gpsimd.tensor_scalar_mul(
                        qs[:, bass.ts(m, 128)], qt[:, bass.ts(m, 128)], ct[:, m:m + 1])
                else:
                    nc.vector.tensor_scalar_mul(
                        qs[:, bass.ts(m, 128)], qt[:, bass.ts(m, 128)], ct[:, m:m + 1])
            qs_t.append(qs)

        # transpose scaled q to feature-major: qsT[:, m*CHUNK + t*128 ...]
        # tp holds TWO m-blocks -> one [128, 1024] copy per pair
        qsT = qsT_pool.tile([128, NB * CHUNK], DT.bfloat16, tag="qsT")
        for jp in range(NB // 2):
            tp = tp_ps.tile([128, 2 * CHUNK], DT.bfloat16, tag="tp")
            for half in range(2):
                m = 2 * jp + half
                for t in range(TPC):
                    nc.tensor.transpose(
                        tp[:, half * CHUNK + t * 128:half * CHUNK + t * 128 + 128],
                        qs_t[t][:, bass.ts(m, 128)], ident_bf[:])
            # split copy between scalar and vector engines
            if (jp * copy_dve) % 8 < copy_dve:
                nc.vector.tensor_copy(qsT[:, bass.ts(jp, 2 * CHUNK)], tp[:])
            else:
                nc.scalar.copy(qsT[:, bass.ts(jp, 2 * CHUNK)], tp[:])
        return qsT

    total_chunks = n_chunks * repeats
    PRE = min(2, total_chunks)
    pending = {}
    for i in range(PRE):
        pending[i] = front((i % n_chunks) * CHUNK)
    # MLP emitted after the first fronts: its PE/ACT work overlaps the
    # initial q DMA + stats + scale + transposes
    with ExitStack() as sctx:
        emit_mlp(sctx)

    for it in range(total_chunks):
        chunk = it % n_chunks
        c0 = chunk * CHUNK
        if it + PRE < total_chunks and it + PRE not in pending:
            pending[it + PRE] = front(((it + PRE) % n_chunks) * CHUNK)
        qsT = pending.pop(it)

        # paired logits + exp: pair j covers m=2j (psum rows 0:64), m=2j+1 (rows 64:128)
        e_j = []
        for j in range(NB // 2):
            lg = lg_ps.tile([128, CHUNK], DT.float32, tag="lg")
            nc.tensor.matmul(
                lg[0:64, :], k2t[:, j * 128:j * 128 + 64],
                qsT[:, bass.ts(2 * j, CHUNK)], start=True, stop=True)
            nc.tensor.matmul(
                lg[64:128, :], k2t[:, j * 128 + 64:j * 128 + 128],
                qsT[:, bass.ts(2 * j + 1, CHUNK)], start=True, stop=True,
                tile_position=(0, 64))
            ej = e_pool.tile([128, CHUNK], DT.bfloat16, tag="ej")
            nc.scalar.activation(ej[:], lg[:], AF.Exp)
            e_j.append(ej)

        # denominators (token-major) -> reciprocal -> transpose -> replicate
        rT_ps = lg_ps.tile([128, CHUNK], DT.float32, tag="lg")
        for t in range(TPC):
            dn = dn_ps.tile([128, 16], DT.float32, tag="dn")
            # denominators: one N=2 matmul per pair, full-K stationary
            for j in range(NB // 2):
                nc.tensor.matmul(
                    dn[:, 2 * j:2 * j + 2], e_j[j][:, bass.ts(t, 128)],
                    ones2_bf[:], start=True, stop=True)
            rct = stat_pool.tile([128, 16], DT.float32, tag="rct")
            nc.vector.reciprocal(rct[:], dn[:])
            nc.tensor.transpose(rT_ps[0:16, bass.ts(t, 128)], rct[:], ident_f32[:])
        rT_sb = o_pool.tile([16, CHUNK], DT.bfloat16, tag="rTsb")
        nc.vector.tensor_copy(rT_sb[:], rT_ps[0:16, :])

        # e-side normalize: ehat_j[p, tok] = e_j[p, tok] * r[m(p), tok].
        # rrep = S_j.T @ rT -- S_j is a 0/1 selection matrix, so the bf16
        # "transpose" matmul (dtype passthrough to PSUM) is exact and lets the
        # DVE multiply run in 2x mode.
        eh_j = []
        for j in range(NB // 2):
            rrep = op_ps.tile([128, CHUNK], DT.bfloat16, tag="op")
            nc.tensor.matmul(
                rrep[:], s_all[:, bass.ts(j, 128)], rT_sb[:],
                start=True, stop=True, is_transpose=True, tile_position=(0, 0))
            eh = e2_pool.tile([128, CHUNK], DT.bfloat16, tag="eh")
            nc.vector.tensor_mul(eh[:], e_j[j][:], rrep[:])
            eh_j.append(eh)

        # out-projection: normalized already -> plain evict copies (cast to bf16).
        # In the drain (last iterations) the fronts are done and DVE idles, so
        # ramp the evict split toward DVE there.
        if it >= total_chunks - 2:
            ev = max(evcopy_dve, 3 if it == total_chunks - 1 else 2)
        else:
            ev = evcopy_dve
        for t in range(TPC):
            osb = o_pool.tile([128, D], DT.bfloat16, tag="osb")
            for g in range(4):
                op = op_ps.tile([128, 512], DT.float32, tag="op")
                for mi in range(4):
                    m = 4 * g + mi
                    j = m // 2
                    krt_x = krt_e if m % 2 == 0 else krt_o
                    nc.tensor.matmul(
                        op[:, bass.ts(mi, 128)], eh_j[j][:, bass.ts(t, 128)],
                        krt_x[:, bass.ts(j, 128)], start=True, stop=True)
                if (g * ev) % 4 < ev:
                    nc.vector.tensor_copy(osb[:, bass.ts(g, 512)], op[:])
                else:
                    nc.scalar.copy(osb[:, bass.ts(g, 512)], op[:])
            nc.gpsimd.dma_start(out_ext[c0 + t * 128:c0 + t * 128 + 128, :], osb[:])


# ---------------------------------------------------------------------------
# host-side wrapper
# ---------------------------------------------------------------------------

_BUILD_CACHE = {}


def _split_multi_waits(nc):
    """walrus here allows at most one semaphore wait per instruction; hoist
    extras onto preceding same-engine NOPs (engine blocks on them in order)."""
    n = 0
    for f in nc.m.functions:
        for blk in f.blocks:
            new = []
            for inst in blk.instructions:
                si = getattr(inst, "sync_info", None)
                if si is not None and si.on_wait and len(si.on_wait) > 1:
                    waits = list(si.on_wait)
                    for w in waits[:-1]:
                        n += 1
                        new.append(mybir.InstNoOp(
                            name=f"{inst.name}_w{n}",
                            ins=[], outs=[],
                            engine=inst.engine,
                            sync_info=mybir.SyncInfo(on_wait=[w], on_update=[]),
                            bass_nofuse=True,
                        ))
                    si.on_wait = [waits[-1]]
                new.append(inst)
            blk.instructions = new
    return n


def _build(n_tokens=N_TOKENS, repeats=1):
    key = (n_tokens, repeats)
    if key in _BUILD_CACHE:
        return _BUILD_CACHE[key]
    nc = bass.Bass("TRN2", target_bir_lowering=False, debug=False, num_devices=N_CORES)
    ins = {
        "q": nc.declare_dram_parameter("q", [n_tokens, D], DT.bfloat16, isOutput=False)[:],
        "mp": nc.declare_dram_parameter("mp", [DB, NB * P], DT.bfloat16, isOutput=False)[:],
        "w1": nc.declare_dram_parameter("w1", [DB, H], DT.bfloat16, isOutput=False)[:],
        "w2": nc.declare_dram_parameter("w2", [H, H], DT.bfloat16, isOutput=False)[:],
        "w3": nc.declare_dram_parameter("w3", [H, H], DT.bfloat16, isOutput=False)[:],
        "w4": nc.declare_dram_parameter("w4", [H, DB], DT.bfloat16, isOutput=False)[:],
        "b123": nc.declare_dram_parameter("b123", [128, 12], DT.float32, isOutput=False)[:],
        "b4r": nc.declare_dram_parameter("b4r", [1, 128], DT.bfloat16, isOutput=False)[:],
        "sall": nc.declare_dram_parameter("sall", [NB, NB // 2 * 128], DT.bfloat16, isOutput=False)[:],
    }
    outs = {
        "out": nc.declare_dram_parameter("out", [n_tokens, D], DT.bfloat16, isOutput=True)[:],
    }
    with ExitStack() as ctx:
        tc = ctx.enter_context(tile.TileContext(nc))
        emit_kernel(ctx, tc, outs, ins, n_tokens=n_tokens, repeats=repeats)
    _split_multi_waits(nc)
    _BUILD_CACHE[key] = nc
    return nc


def _host_prep(queries, mem_params, W1, b1, W2, b2, W3, b3, W4, b4):
    q_bf = np.asarray(queries).astype(ml_dtypes.bfloat16)
    bf = lambda a: np.ascontiguousarray(np.asarray(a, dtype=np.float32)).astype(ml_dtypes.bfloat16)
    # feature-major mem_params, cols ordered (m, p): col = m*64 + p
    mp = np.ascontiguousarray(
        np.asarray(mem_params).reshape(P, NB, DB).transpose(2, 1, 0).reshape(DB, NB * P))
    b123 = np.concatenate(
        [np.asarray(b).reshape(4, 128).T for b in (b1, b2, b3)], axis=1
    ).astype(np.float32)
    b123 = np.ascontiguousarray(b123)
    s_all = np.zeros((NB, NB // 2 * 128), np.float32)
    for j in range(NB // 2):
        s_all[2 * j, j * 128:j * 128 + 64] = 1.0
        s_all[2 * j + 1, j * 128 + 64:j * 128 + 128] = 1.0
    common = {
        "sall": s_all.astype(ml_dtypes.bfloat16),
        "mp": bf(mp),
        "w1": bf(W1),
        "w2": bf(W2),
        "w3": bf(W3),
        "w4": bf(W4),
        "b123": b123,
        "b4r": bf(np.asarray(b4).reshape(1, 128)),
    }
    in_maps = []
    for b in range(N_CORES):
        m = dict(common)
        m["q"] = np.ascontiguousarray(q_bf[b])
        in_maps.append(m)
    return in_maps


def kernel(queries, mem_params, W1, b1, W2, b2, W3, b3, W4, b4):
    nc = _build(N_TOKENS)
    in_maps = _host_prep(queries, mem_params, W1, b1, W2, b2, W3, b3, W4, b4)
    trace = bool(int(os.environ.get("KERNEL_TRACE", "0")))
    try:
        res = run_bass_kernel_spmd(nc, in_maps, list(range(N_CORES)), trace=trace)
    except ModuleNotFoundError:
        res = run_bass_kernel_spmd(nc, in_maps, list(range(N_CORES)), trace=False)
    kernel.last_exec_time_ns = res.exec_time_ns
    kernel.last_results = res
    out = np.stack([res.results[i]["out"] for i in range(N_CORES)], axis=0)
    return out.astype(np.float32)


kernel.last_exec_time_ns = None



# revision 45
# speedup vs baseline: 1.1663x; 1.1663x over previous
"""Trainium2 Bass kernel for BlockPrototypeMemory (sparse block attention).

Computation (reference):
  mem = MLP(mem_params)            # (P=64, NB=16, DB=128) rows through 128->512->512->512->128 MLP
  khat = block_ln(mem)             # LayerNorm per (p, m) row over DB
  qhat = block_ln(queries)         # LayerNorm per (token, m) block over DB
  logits[b,m,n,p] = qhat . khat / sqrt(DB)
  out = softmax_p(logits) @ khat

Key algebraic trick: khat rows are exactly zero-mean over DB (LayerNorm output), so
  logits = (1/(sigma_q sqrt(DB))) * (q_raw . khat)
i.e. q's mean subtraction is unnecessary; only the per-(token,block) scale
c = 1/sqrt((var+eps)*DB) must be applied to q before the matmul.

Sharding: data-parallel over B (8 batches -> 8 cores), MLP + mem replicated per core.
"""

import os
import sys

sys.path.insert(0, "/opt/trn_rl_repo")

import numpy as np
import ml_dtypes
from contextlib import ExitStack

from concourse import bass, mybir, tile, masks
from concourse.bass_utils import run_bass_kernel_spmd

AF = mybir.ActivationFunctionType
ALU = mybir.AluOpType
DT = mybir.dt

P, NB, D, DB, H = 64, 16, 2048, 128, 512
EPS = 1e-5
N_CORES = 8
N_TOKENS = 4096
CHUNK = 512          # tokens per macro-iteration
TPC = CHUNK // 128   # 128-token tiles per chunk

def emit_kernel(ctx: ExitStack, tc: "tile.TileContext", outs, ins, n_tokens=N_TOKENS,
                repeats=1):
    """Emit the per-core kernel. ins/outs are dicts of DRAM APs."""
    nc = tc.nc
    q_ext = ins["q"]          # [n_tokens, D] bf16   (token-major)
    mp_ext = ins["mp"]        # [DB, NB*P] bf16  feature-major, cols ordered (m, p)
    w1_ext = ins["w1"]        # [DB, H] bf16
    w2_ext = ins["w2"]        # [H, H] bf16
    w3_ext = ins["w3"]        # [H, H] bf16
    w4_ext = ins["w4"]        # [H, DB] bf16
    b123_ext = ins["b123"]    # [128, 12] f32 (b1|b2|b3 each reshaped (4,128).T)
    b4r_ext = ins["b4r"]      # [1, 128] bf16
    out_ext = outs["out"]     # [n_tokens, D] f32

    n_chunks = n_tokens // CHUNK
    NROW = NB * P            # 1024 rows through the MLP
    NRT = NROW // 128        # 8 row-tiles

    # ---------------- constant tiles ----------------
    const_pool = ctx.enter_context(tc.tile_pool(name="const", bufs=1))
    ident_bf = const_pool.tile([128, 128], DT.bfloat16, tag="identbf")
    ident_f32 = const_pool.tile([128, 128], DT.float32, tag="identf32")
    masks.make_identity(nc, ident_bf[:])
    masks.make_identity(nc, ident_f32[:])
    ones2_bf = const_pool.tile([128, 2], DT.bfloat16, tag="ones2")
    nc.gpsimd.memset(ones2_bf[:], 0.0)
    nc.gpsimd.memset(ones2_bf[0:64, 0:1], 1.0)
    nc.gpsimd.memset(ones2_bf[64:128, 1:2], 1.0)
    epsb = const_pool.tile([128, 1], DT.float32, tag="epsb")
    nc.gpsimd.memset(epsb[:], float(128.0 * EPS))
    # S_all[:, j*128:(j+1)*128] is the [16,128] stationary that replicates
    # rT rows (2j, 2j+1) onto partition halves (0:64, 64:128) via matmul
    s_all = const_pool.tile([16, NB // 2 * 128], DT.bfloat16, tag="sall")
    nc.sync.dma_start(s_all[:], ins["sall"][:, :])

    # persistent khat tiles
    khat_pool = ctx.enter_context(tc.tile_pool(name="khat", bufs=1))
    # k2t: transposed khat, feature-major per pair: cols [ro*128 : ro*128+64] = m=2ro, next 64 = m=2ro+1
    k2t = khat_pool.tile([128, NRT * 128], DT.bfloat16, tag="k2t")
    # krt_eo: row-major khat for pair j at cols [j*256, (j+1)*256): first 128
    # cols = khat for m=2j (partition rows 64:128 zeroed), next 128 = m=2j+1
    # (rows 0:64 zeroed).  One 256-col moving operand covers a whole pair.
    krt_eo = khat_pool.tile([128, NRT * 256], DT.bfloat16, tag="krteo")
    nc.gpsimd.memset(krt_eo[:], 0.0)

    # ---------------- MLP + khat setup (bf16 matmuls, fp32 LN) ----------------
    def emit_mlp(sctx):
        mw = sctx.enter_context(tc.tile_pool(name="mlpw", bufs=1))
        mact = sctx.enter_context(tc.tile_pool(name="mlpact", bufs=1))
        msc = sctx.enter_context(tc.tile_pool(name="mlpsc", bufs=2))

        w1_sb = mw.tile([128, H], DT.bfloat16, tag="w1")
        nc.sync.dma_start(w1_sb[:], w1_ext[:, :])
        w2_sb = mw.tile([128, 4 * H], DT.bfloat16, tag="w2")
        w3_sb = mw.tile([128, 4 * H], DT.bfloat16, tag="w3")
        for ki in range(4):
            nc.sync.dma_start(w2_sb[:, bass.ts(ki, H)], w2_ext[bass.ts(ki, 128), :])
            nc.sync.dma_start(w3_sb[:, bass.ts(ki, H)], w3_ext[bass.ts(ki, 128), :])
        w4_sb = mw.tile([128, 4 * 128], DT.bfloat16, tag="w4")
        for ki in range(4):
            nc.sync.dma_start(w4_sb[:, bass.ts(ki, 128)], w4_ext[bass.ts(ki, 128), :])
        b123_sb = mw.tile([128, 12], DT.float32, tag="b123")
        nc.sync.dma_start(b123_sb[:], b123_ext[:, :])
        b4r_sb = mw.tile([1, 128], DT.bfloat16, tag="b4r")
        nc.sync.dma_start(b4r_sb[:], b4r_ext[:, :])
        ones_row_bf = mw.tile([1, 128], DT.bfloat16, tag="onesrowbf")
        nc.gpsimd.memset(ones_row_bf[:], 1.0)

        # mem_params arrive host-transposed (feature-major) in bf16
        x_fm = mact.tile([128, NROW], DT.bfloat16, tag="xfm")
        nc.sync.dma_start(x_fm[:], mp_ext[:, :])

        # L1: h1[mo] = relu(W1[:,mo].T @ x + b1[mo])   feature-major [128, NROW] x4
        h1 = mact.tile([128, 4 * NROW], DT.bfloat16, tag="h1")
        for mo in range(4):
            for ch in range(NROW // 512):
                ps = lg_ps.tile([128, 512], DT.float32, tag="lg")
                nc.tensor.matmul(
                    ps[:], w1_sb[:, bass.ts(mo, 128)],
                    x_fm[:, bass.ts(ch, 512)], start=True, stop=True)
                nc.scalar.activation(
                    h1[:, mo * NROW + ch * 512:mo * NROW + ch * 512 + 512], ps[:],
                    AF.Relu, bias=b123_sb[:, mo:mo + 1])
        # L2 / L3 (L3 reuses h1's buffer: h1 is dead once L2 is done)
        hprev = h1
        for li, (w_sb, boff) in enumerate([(w2_sb, 4), (w3_sb, 8)]):
            hnext = mact.tile([128, 4 * NROW], DT.bfloat16, tag=["h2", "h1"][li])
            for mo in range(4):
                for ch in range(NROW // 512):
                    ps = lg_ps.tile([128, 512], DT.float32, tag="lg")
                    for ki in range(4):
                        nc.tensor.matmul(
                            ps[:],
                            w_sb[:, ki * H + mo * 128:ki * H + mo * 128 + 128],
                            hprev[:, ki * NROW + ch * 512:ki * NROW + ch * 512 + 512],
                            start=(ki == 0), stop=(ki == 3))
                    nc.scalar.activation(
                        hnext[:, mo * NROW + ch * 512:mo * NROW + ch * 512 + 512], ps[:],
                        AF.Relu, bias=b123_sb[:, boff + mo:boff + mo + 1])
            hprev = hnext

        # L4 (token-major out) + bias via rank-1 + block LN -> khat
        for ro in range(NRT):
            ps4 = dn_ps.tile([128, 128], DT.float32, tag="dn")
            for ki in range(4):
                nc.tensor.matmul(
                    ps4[:],
                    hprev[:, ki * NROW + ro * 128:ki * NROW + ro * 128 + 128],
                    w4_sb[:, bass.ts(ki, 128)],
                    start=(ki == 0), stop=False)
            nc.tensor.matmul(ps4[:], ones_row_bf[:], b4r_sb[:],
                             start=False, stop=True)
            # row LayerNorm stats (over 128 features)
            st = msc.tile([128, 6], DT.float32, tag="mst")
            nc.vector.bn_stats(st[:], ps4[:])
            mean = msc.tile([128, 1], DT.float32, tag="mmean")
            var = msc.tile([128, 1], DT.float32, tag="mvar")
            tmp = msc.tile([128, 1], DT.float32, tag="mtmp")
            nc.vector.tensor_add(mean[:], st[:, 1:2], st[:, 4:5])
            nc.vector.tensor_scalar_mul(mean[:], mean[:], 0.5)
            nc.vector.tensor_add(var[:], st[:, 2:3], st[:, 5:6])
            nc.vector.tensor_sub(tmp[:], st[:, 1:2], st[:, 4:5])
            nc.vector.tensor_mul(tmp[:], tmp[:], tmp[:])
            nc.vector.tensor_scalar(var[:], var[:], 1.0 / 128.0, None, ALU.mult)
            nc.vector.tensor_scalar_mul(tmp[:], tmp[:], 0.25)
            nc.vector.tensor_add(var[:], var[:], tmp[:])
            # sc = rsqrt(var+eps) = exp(-0.5*ln(var+eps)); nbias = -mean*sc
            sc = msc.tile([128, 1], DT.float32, tag="msc")
            nc.vector.tensor_scalar_add(sc[:], var[:], EPS)
            nc.scalar.activation(sc[:], sc[:], AF.Ln)
            nc.scalar.activation(sc[:], sc[:], AF.Exp, scale=-0.5)
            nbias = msc.tile([128, 1], DT.float32, tag="mnb")
            nc.vector.tensor_mul(nbias[:], mean[:], sc[:])
            nc.vector.tensor_scalar_mul(nbias[:], nbias[:], -1.0)
            ktm = msc.tile([128, 128], DT.float32, tag="ktm")
            nc.scalar.activation(ktm[:], ps4[:], AF.Identity, bias=nbias[:], scale=sc[:])
            # bf16 masked row-major copies + transposed copy
            nc.scalar.copy(krt_eo[0:64, ro * 256:ro * 256 + 128], ktm[0:64, :])
            nc.scalar.copy(krt_eo[64:128, ro * 256 + 128:ro * 256 + 256], ktm[64:128, :])
            ptk = dn_ps.tile([128, 128], DT.float32, tag="dn")
            nc.tensor.transpose(ptk[:], ktm[:], ident_f32[:])
            nc.scalar.copy(k2t[:, bass.ts(ro, 128)], ptk[:])

    # ---------------- attention over token chunks ----------------
    q_pool = ctx.enter_context(tc.tile_pool(name="qin", bufs=10))
    qs_pool = ctx.enter_context(tc.tile_pool(name="qs", bufs=8))
    qsT_pool = ctx.enter_context(tc.tile_pool(name="qsT", bufs=3))
    e_pool = ctx.enter_context(tc.tile_pool(name="eexp", bufs=10))
    e2_pool = ctx.enter_context(tc.tile_pool(name="ehat", bufs=10))
    o_pool = ctx.enter_context(tc.tile_pool(name="osb", bufs=4))
    stat_pool = ctx.enter_context(tc.tile_pool(name="stat", bufs=10))
    tp_ps = ctx.enter_context(tc.tile_pool(name="tp_ps", bufs=2, space="PSUM"))
    lg_ps = ctx.enter_context(tc.tile_pool(name="lg_ps", bufs=2, space="PSUM"))
    op_ps = ctx.enter_context(tc.tile_pool(name="op_ps", bufs=2, space="PSUM"))
    dn_ps = ctx.enter_context(tc.tile_pool(name="dn_ps", bufs=2, space="PSUM"))

    qscale_pool = int(os.environ.get("QSCALE_POOL", "13"))
    copy_dve = int(os.environ.get("COPY_DVE", "4"))
    evcopy_dve = int(os.environ.get("EVCOPY_DVE", "0"))

    def front(c0):
        """DMA + stats + scale + transpose for one chunk -> qsT tile."""
        q_t = []
        qs_t = []
        for t in range(TPC):
            qt = q_pool.tile([128, D], DT.bfloat16, tag="qt")
            nc.sync.dma_start(qt[:], q_ext[c0 + t * 128:c0 + t * 128 + 128, :])
            q_t.append(qt)
        for t in range(TPC):
            qt = q_t[t]
            st = stat_pool.tile([128, 16, 6], DT.float32, tag="st")
            # HW BNStats requires exactly 6 outputs/partition -> one instr per block
            for m in range(NB):
                nc.vector.bn_stats(st[:, m, :], qt[:, bass.ts(m, 128)])
            # V = 128*var = (st2+st5) + 32*(st1-st4)^2
            var = stat_pool.tile([128, 16], DT.float32, tag="var")
            tmp = stat_pool.tile([128, 16], DT.float32, tag="tmp")
            nc.vector.tensor_sub(tmp[:], st[:, :, 1], st[:, :, 4])
            nc.vector.scalar_tensor_tensor(tmp[:], tmp[:], 32.0, tmp[:], ALU.mult, ALU.mult)
            nc.vector.tensor_add(var[:], st[:, :, 2], st[:, :, 5])
            nc.vector.tensor_add(var[:], var[:], tmp[:])
            # c = rsqrt((var+eps)*DB) = exp(-0.5*ln(V + DB*eps))
            ct = stat_pool.tile([128, 16], DT.float32, tag="ct")
            nc.scalar.activation(ct[:], var[:], AF.Ln, bias=epsb[:])
            nc.scalar.activation(ct[:], ct[:], AF.Exp, scale=-0.5)
            qs = qs_pool.tile([128, D], DT.bfloat16, tag="qst")
            for m in range(NB):
                if (m * qscale_pool) % 16 < qscale_pool:
                    nc.gpsimd.tensor_scalar_mul(
                        qs[:, bass.ts(m, 128)], qt[:, bass.ts(m, 128)], ct[:, m:m + 1])
                else:
                    nc.vector.tensor_scalar_mul(
                        qs[:, bass.ts(m, 128)], qt[:, bass.ts(m, 128)], ct[:, m:m + 1])
            qs_t.append(qs)

        # transpose scaled q to feature-major: qsT[:, m*CHUNK + t*128 ...]
        # tp holds TWO m-blocks -> one [128, 1024] copy per pair
        qsT = qsT_pool.tile([128, NB * CHUNK], DT.bfloat16, tag="qsT")
        for jp in range(NB // 2):
            tp = tp_ps.tile([128, 2 * CHUNK], DT.bfloat16, tag="tp")
            for half in range(2):
                m = 2 * jp + half
                for t in range(TPC):
                    nc.tensor.transpose(
                        tp[:, half * CHUNK + t * 128:half * CHUNK + t * 128 + 128],
                        qs_t[t][:, bass.ts(m, 128)], ident_bf[:])
            # split copy between scalar and vector engines
            if (jp * copy_dve) % 8 < copy_dve:
                nc.vector.tensor_copy(qsT[:, bass.ts(jp, 2 * CHUNK)], tp[:])
            else:
                nc.scalar.copy(qsT[:, bass.ts(jp, 2 * CHUNK)], tp[:])
        return qsT

    total_chunks = n_chunks * repeats
    PRE = min(2, total_chunks)
    pending = {}
    for i in range(PRE):
        pending[i] = front((i % n_chunks) * CHUNK)
    # MLP emitted after the first fronts: its PE/ACT work overlaps the
    # initial q DMA + stats + scale + transposes
    with ExitStack() as sctx:
        emit_mlp(sctx)

    for it in range(total_chunks):
        chunk = it % n_chunks
        c0 = chunk * CHUNK
        if it + PRE < total_chunks and it + PRE not in pending:
            pending[it + PRE] = front(((it + PRE) % n_chunks) * CHUNK)
        qsT = pending.pop(it)

        # paired logits + exp: pair j covers m=2j (psum rows 0:64), m=2j+1 (rows 64:128)
        e_j = []
        for j in range(NB // 2):
            lg = lg_ps.tile([128, CHUNK], DT.float32, tag="lg")
            nc.tensor.matmul(
                lg[0:64, :], k2t[:, j * 128:j * 128 + 64],
                qsT[:, bass.ts(2 * j, CHUNK)], start=True, stop=True)
            nc.tensor.matmul(
                lg[64:128, :], k2t[:, j * 128 + 64:j * 128 + 128],
                qsT[:, bass.ts(2 * j + 1, CHUNK)], start=True, stop=True,
                tile_position=(0, 64))
            ej = e_pool.tile([128, CHUNK], DT.bfloat16, tag="ej")
            nc.scalar.activation(ej[:], lg[:], AF.Exp)
            e_j.append(ej)

        # denominators (token-major) -> reciprocal -> transpose -> replicate
        rT_ps = lg_ps.tile([128, CHUNK], DT.float32, tag="lg")
        for t in range(TPC):
            dn = dn_ps.tile([128, 16], DT.float32, tag="dn")
            # denominators: one N=2 matmul per pair, full-K stationary
            for j in range(NB // 2):
                nc.tensor.matmul(
                    dn[:, 2 * j:2 * j + 2], e_j[j][:, bass.ts(t, 128)],
                    ones2_bf[:], start=True, stop=True)
            rct = stat_pool.tile([128, 16], DT.float32, tag="rct")
            nc.vector.reciprocal(rct[:], dn[:])
            nc.tensor.transpose(rT_ps[0:16, bass.ts(t, 128)], rct[:], ident_f32[:])
        rT_sb = o_pool.tile([16, CHUNK], DT.bfloat16, tag="rTsb")
        nc.vector.tensor_copy(rT_sb[:], rT_ps[0:16, :])

        # e-side normalize: ehat_j[p, tok] = e_j[p, tok] * r[m(p), tok].
        # rrep = S_j.T @ rT -- S_j is a 0/1 selection matrix, so the bf16
        # "transpose" matmul (dtype passthrough to PSUM) is exact and lets the
        # DVE multiply run in 2x mode.
        eh_j = []
        for j in range(NB // 2):
            rrep = op_ps.tile([128, CHUNK], DT.bfloat16, tag="op")
            nc.tensor.matmul(
                rrep[:], s_all[:, bass.ts(j, 128)], rT_sb[:],
                start=True, stop=True, is_transpose=True, tile_position=(0, 0))
            eh = e2_pool.tile([128, CHUNK], DT.bfloat16, tag="eh")
            nc.vector.tensor_mul(eh[:], e_j[j][:], rrep[:])
            eh_j.append(eh)

        # out-projection: normalized already -> plain evict copies (cast to bf16).
        # In the drain (last iterations) the fronts are done and DVE idles, so
        # ramp the evict split toward DVE there.
        if it >= total_chunks - 2:
            ev = max(evcopy_dve, 3 if it == total_chunks - 1 else 2)
        else:
            ev = evcopy_dve
        for t in range(TPC):
            osb = o_pool.tile([128, D], DT.bfloat16, tag="osb")
            for g in range(4):
                op = op_ps.tile([128, 512], DT.float32, tag="op")
                for ji in range(2):
                    j = 2 * g + ji
                    nc.tensor.matmul(
                        op[:, bass.ts(ji, 256)], eh_j[j][:, bass.ts(t, 128)],
                        krt_eo[:, bass.ts(j, 256)], start=True, stop=True)
                if (g * ev) % 4 < ev:
                    nc.vector.tensor_copy(osb[:, bass.ts(g, 512)], op[:])
                else:
                    nc.scalar.copy(osb[:, bass.ts(g, 512)], op[:])
            nc.gpsimd.dma_start(out_ext[c0 + t * 128:c0 + t * 128 + 128, :], osb[:])


# ---------------------------------------------------------------------------
# host-side wrapper
# ---------------------------------------------------------------------------

_BUILD_CACHE = {}


def _split_multi_waits(nc):
    """walrus here allows at most one semaphore wait per instruction; hoist
    extras onto preceding same-engine NOPs (engine blocks on them in order)."""
    n = 0
    for f in nc.m.functions:
        for blk in f.blocks:
            new = []
            for inst in blk.instructions:
                si = getattr(inst, "sync_info", None)
                if si is not None and si.on_wait and len(si.on_wait) > 1:
                    waits = list(si.on_wait)
                    for w in waits[:-1]:
                        n += 1
                        new.append(mybir.InstNoOp(
                            name=f"{inst.name}_w{n}",
                            ins=[], outs=[],
                            engine=inst.engine,
                            sync_info=mybir.SyncInfo(on_wait=[w], on_update=[]),
                            bass_nofuse=True,
                        ))
                    si.on_wait = [waits[-1]]
                new.append(inst)
            blk.instructions = new
    return n


def _build(n_tokens=N_TOKENS, repeats=1):
    key = (n_tokens, repeats)
    if key in _BUILD_CACHE:
        return _BUILD_CACHE[key]
    nc = bass.Bass("TRN2", target_bir_lowering=False, debug=False, num_devices=N_CORES)
    ins = {
        "q": nc.declare_dram_parameter("q", [n_tokens, D], DT.bfloat16, isOutput=False)[:],
        "mp": nc.declare_dram_parameter("mp", [DB, NB * P], DT.bfloat16, isOutput=False)[:],
        "w1": nc.declare_dram_parameter("w1", [DB, H], DT.bfloat16, isOutput=False)[:],
        "w2": nc.declare_dram_parameter("w2", [H, H], DT.bfloat16, isOutput=False)[:],
        "w3": nc.declare_dram_parameter("w3", [H, H], DT.bfloat16, isOutput=False)[:],
        "w4": nc.declare_dram_parameter("w4", [H, DB], DT.bfloat16, isOutput=False)[:],
        "b123": nc.declare_dram_parameter("b123", [128, 12], DT.float32, isOutput=False)[:],
        "b4r": nc.declare_dram_parameter("b4r", [1, 128], DT.bfloat16, isOutput=False)[:],
        "sall": nc.declare_dram_parameter("sall", [NB, NB // 2 * 128], DT.bfloat16, isOutput=False)[:],
    }
    outs = {
        "out": nc.declare_dram_parameter("out", [n_tokens, D], DT.bfloat16, isOutput=True)[:],
    }
    with ExitStack() as ctx:
        tc = ctx.enter_context(tile.TileContext(nc))
        emit_kernel(ctx, tc, outs, ins, n_tokens=n_tokens, repeats=repeats)
    _split_multi_waits(nc)
    _BUILD_CACHE[key] = nc
    return nc


def _host_prep(queries, mem_params, W1, b1, W2, b2, W3, b3, W4, b4):
    q_bf = np.asarray(queries).astype(ml_dtypes.bfloat16)
    bf = lambda a: np.ascontiguousarray(np.asarray(a, dtype=np.float32)).astype(ml_dtypes.bfloat16)
    # feature-major mem_params, cols ordered (m, p): col = m*64 + p
    mp = np.ascontiguousarray(
        np.asarray(mem_params).reshape(P, NB, DB).transpose(2, 1, 0).reshape(DB, NB * P))
    b123 = np.concatenate(
        [np.asarray(b).reshape(4, 128).T for b in (b1, b2, b3)], axis=1
    ).astype(np.float32)
    b123 = np.ascontiguousarray(b123)
    s_all = np.zeros((NB, NB // 2 * 128), np.float32)
    for j in range(NB // 2):
        s_all[2 * j, j * 128:j * 128 + 64] = 1.0
        s_all[2 * j + 1, j * 128 + 64:j * 128 + 128] = 1.0
    common = {
        "sall": s_all.astype(ml_dtypes.bfloat16),
        "mp": bf(mp),
        "w1": bf(W1),
        "w2": bf(W2),
        "w3": bf(W3),
        "w4": bf(W4),
        "b123": b123,
        "b4r": bf(np.asarray(b4).reshape(1, 128)),
    }
    in_maps = []
    for b in range(N_CORES):
        m = dict(common)
        m["q"] = np.ascontiguousarray(q_bf[b])
        in_maps.append(m)
    return in_maps


def kernel(queries, mem_params, W1, b1, W2, b2, W3, b3, W4, b4):
    nc = _build(N_TOKENS)
    in_maps = _host_prep(queries, mem_params, W1, b1, W2, b2, W3, b3, W4, b4)
    trace = bool(int(os.environ.get("KERNEL_TRACE", "0")))
    try:
        res = run_bass_kernel_spmd(nc, in_maps, list(range(N_CORES)), trace=trace)
    except ModuleNotFoundError:
        res = run_bass_kernel_spmd(nc, in_maps, list(range(N_CORES)), trace=False)
    kernel.last_exec_time_ns = res.exec_time_ns
    kernel.last_results = res
    out = np.stack([res.results[i]["out"] for i in range(N_CORES)], axis=0)
    return out.astype(np.float32)


kernel.last_exec_time_ns = None



# revision 48
# speedup vs baseline: 15.8082x; 13.5540x over previous
"""Trainium2 Bass kernel for BlockPrototypeMemory (sparse block attention).

Computation (reference):
  mem = MLP(mem_params)            # (P=64, NB=16, DB=128) rows through 128->512->512->512->128 MLP
  khat = block_ln(mem)             # LayerNorm per (p, m) row over DB
  qhat = block_ln(queries)         # LayerNorm per (token, m) block over DB
  logits[b,m,n,p] = qhat . khat / sqrt(DB)
  out = softmax_p(logits) @ khat

Key tricks:
- khat rows are exactly zero-mean over DB (LayerNorm output), so q's mean
  subtraction cancels in the logits; only the per-(token,block) inverse-sigma
  scale c = 1/sqrt((var+eps)*DB) is applied to q before the matmul.
- softmax normalization is applied on the exp side (ehat = e * r with
  r = 1/sum_p e, replicated across partition halves by a 0/1 selection
  "transpose" matmul in bf16), so the out-projection PSUM only needs a plain
  cast-copy eviction instead of 512 small per-block scaled evictions.
- per-chunk fronts (DMA + bn_stats + var + scale + PE transposes) are
  software-pipelined two chunks ahead and overlap the one-time MLP/khat setup.
- output is written bf16 (halves out-DMA traffic; ~1e-3 extra rel-err).
- GPSIMD offload of the q-scale was tried and REVERTED: the DVE<->GpSimd
  shared SBUF port is an exclusive lock on HW, and Pool tensor ops made the
  whole kernel ~3x slower (unmodeled by the TimelineSim cost model).

Sharding: data-parallel over B (8 batches -> 8 cores), MLP + mem replicated per core.
"""

import os
import sys

sys.path.insert(0, "/opt/trn_rl_repo")

import numpy as np
import ml_dtypes
from contextlib import ExitStack

from concourse import bass, mybir, tile, masks
from concourse.bass_utils import run_bass_kernel_spmd

AF = mybir.ActivationFunctionType
ALU = mybir.AluOpType
DT = mybir.dt

P, NB, D, DB, H = 64, 16, 2048, 128, 512
EPS = 1e-5
N_CORES = 8
N_TOKENS = 4096
CHUNK = 512          # tokens per macro-iteration
TPC = CHUNK // 128   # 128-token tiles per chunk

def emit_kernel(ctx: ExitStack, tc: "tile.TileContext", outs, ins, n_tokens=N_TOKENS,
                repeats=1):
    """Emit the per-core kernel. ins/outs are dicts of DRAM APs."""
    nc = tc.nc
    q_ext = ins["q"]          # [n_tokens, D] bf16   (token-major)
    mp_ext = ins["mp"]        # [DB, NB*P] bf16  feature-major, cols ordered (m, p)
    w1_ext = ins["w1"]        # [DB, H] bf16
    w2_ext = ins["w2"]        # [H, H] bf16
    w3_ext = ins["w3"]        # [H, H] bf16
    w4_ext = ins["w4"]        # [H, DB] bf16
    b123_ext = ins["b123"]    # [128, 12] f32 (b1|b2|b3 each reshaped (4,128).T)
    b4r_ext = ins["b4r"]      # [1, 128] bf16
    out_ext = outs["out"]     # [n_tokens, D] f32

    n_chunks = n_tokens // CHUNK
    NROW = NB * P            # 1024 rows through the MLP
    NRT = NROW // 128        # 8 row-tiles

    # ---------------- constant tiles ----------------
    const_pool = ctx.enter_context(tc.tile_pool(name="const", bufs=1))
    ident_bf = const_pool.tile([128, 128], DT.bfloat16, tag="identbf")
    ident_f32 = const_pool.tile([128, 128], DT.float32, tag="identf32")
    masks.make_identity(nc, ident_bf[:])
    masks.make_identity(nc, ident_f32[:])
    ones2_bf = const_pool.tile([128, 2], DT.bfloat16, tag="ones2")
    nc.gpsimd.memset(ones2_bf[:], 0.0)
    nc.gpsimd.memset(ones2_bf[0:64, 0:1], 1.0)
    nc.gpsimd.memset(ones2_bf[64:128, 1:2], 1.0)
    epsb = const_pool.tile([128, 1], DT.float32, tag="epsb")
    nc.gpsimd.memset(epsb[:], float(128.0 * EPS))
    # S_all[:, j*128:(j+1)*128] is the [16,128] stationary that replicates
    # rT rows (2j, 2j+1) onto partition halves (0:64, 64:128) via matmul
    s_all = const_pool.tile([16, NB // 2 * 128], DT.bfloat16, tag="sall")
    nc.sync.dma_start(s_all[:], ins["sall"][:, :])

    # persistent khat tiles
    khat_pool = ctx.enter_context(tc.tile_pool(name="khat", bufs=1))
    # k2t: transposed khat, feature-major per pair: cols [ro*128 : ro*128+64] = m=2ro, next 64 = m=2ro+1
    k2t = khat_pool.tile([128, NRT * 128], DT.bfloat16, tag="k2t")
    # krt_eo: row-major khat for pair j at cols [j*256, (j+1)*256): first 128
    # cols = khat for m=2j (partition rows 64:128 zeroed), next 128 = m=2j+1
    # (rows 0:64 zeroed).  One 256-col moving operand covers a whole pair.
    krt_eo = khat_pool.tile([128, NRT * 256], DT.bfloat16, tag="krteo")
    nc.gpsimd.memset(krt_eo[:], 0.0)

    # ---------------- MLP + khat setup (bf16 matmuls, fp32 LN) ----------------
    def emit_mlp(sctx):
        mw = sctx.enter_context(tc.tile_pool(name="mlpw", bufs=1))
        mact = sctx.enter_context(tc.tile_pool(name="mlpact", bufs=1))
        msc = sctx.enter_context(tc.tile_pool(name="mlpsc", bufs=2))

        w1_sb = mw.tile([128, H], DT.bfloat16, tag="w1")
        nc.sync.dma_start(w1_sb[:], w1_ext[:, :])
        w2_sb = mw.tile([128, 4 * H], DT.bfloat16, tag="w2")
        w3_sb = mw.tile([128, 4 * H], DT.bfloat16, tag="w3")
        for ki in range(4):
            nc.sync.dma_start(w2_sb[:, bass.ts(ki, H)], w2_ext[bass.ts(ki, 128), :])
            nc.sync.dma_start(w3_sb[:, bass.ts(ki, H)], w3_ext[bass.ts(ki, 128), :])
        w4_sb = mw.tile([128, 4 * 128], DT.bfloat16, tag="w4")
        for ki in range(4):
            nc.sync.dma_start(w4_sb[:, bass.ts(ki, 128)], w4_ext[bass.ts(ki, 128), :])
        b123_sb = mw.tile([128, 12], DT.float32, tag="b123")
        nc.sync.dma_start(b123_sb[:], b123_ext[:, :])
        b4r_sb = mw.tile([1, 128], DT.bfloat16, tag="b4r")
        nc.sync.dma_start(b4r_sb[:], b4r_ext[:, :])
        ones_row_bf = mw.tile([1, 128], DT.bfloat16, tag="onesrowbf")
        nc.gpsimd.memset(ones_row_bf[:], 1.0)

        # mem_params arrive host-transposed (feature-major) in bf16
        x_fm = mact.tile([128, NROW], DT.bfloat16, tag="xfm")
        nc.sync.dma_start(x_fm[:], mp_ext[:, :])

        # L1: h1[mo] = relu(W1[:,mo].T @ x + b1[mo])   feature-major [128, NROW] x4
        h1 = mact.tile([128, 4 * NROW], DT.bfloat16, tag="h1")
        for mo in range(4):
            for ch in range(NROW // 512):
                ps = lg_ps.tile([128, 512], DT.float32, tag="lg")
                nc.tensor.matmul(
                    ps[:], w1_sb[:, bass.ts(mo, 128)],
                    x_fm[:, bass.ts(ch, 512)], start=True, stop=True)
                nc.scalar.activation(
                    h1[:, mo * NROW + ch * 512:mo * NROW + ch * 512 + 512], ps[:],
                    AF.Relu, bias=b123_sb[:, mo:mo + 1])
        # L2 / L3 (L3 reuses h1's buffer: h1 is dead once L2 is done)
        hprev = h1
        for li, (w_sb, boff) in enumerate([(w2_sb, 4), (w3_sb, 8)]):
            hnext = mact.tile([128, 4 * NROW], DT.bfloat16, tag=["h2", "h1"][li])
            for mo in range(4):
                for ch in range(NROW // 512):
                    ps = lg_ps.tile([128, 512], DT.float32, tag="lg")
                    for ki in range(4):
                        nc.tensor.matmul(
                            ps[:],
                            w_sb[:, ki * H + mo * 128:ki * H + mo * 128 + 128],
                            hprev[:, ki * NROW + ch * 512:ki * NROW + ch * 512 + 512],
                            start=(ki == 0), stop=(ki == 3))
                    nc.scalar.activation(
                        hnext[:, mo * NROW + ch * 512:mo * NROW + ch * 512 + 512], ps[:],
                        AF.Relu, bias=b123_sb[:, boff + mo:boff + mo + 1])
            hprev = hnext

        # L4 (token-major out) + bias via rank-1 + block LN -> khat
        for ro in range(NRT):
            ps4 = dn_ps.tile([128, 128], DT.float32, tag="dn")
            for ki in range(4):
                nc.tensor.matmul(
                    ps4[:],
                    hprev[:, ki * NROW + ro * 128:ki * NROW + ro * 128 + 128],
                    w4_sb[:, bass.ts(ki, 128)],
                    start=(ki == 0), stop=False)
            nc.tensor.matmul(ps4[:], ones_row_bf[:], b4r_sb[:],
                             start=False, stop=True)
            # row LayerNorm stats (over 128 features)
            st = msc.tile([128, 6], DT.float32, tag="mst")
            nc.vector.bn_stats(st[:], ps4[:])
            mean = msc.tile([128, 1], DT.float32, tag="mmean")
            var = msc.tile([128, 1], DT.float32, tag="mvar")
            tmp = msc.tile([128, 1], DT.float32, tag="mtmp")
            nc.vector.tensor_add(mean[:], st[:, 1:2], st[:, 4:5])
            nc.vector.tensor_scalar_mul(mean[:], mean[:], 0.5)
            nc.vector.tensor_add(var[:], st[:, 2:3], st[:, 5:6])
            nc.vector.tensor_sub(tmp[:], st[:, 1:2], st[:, 4:5])
            nc.vector.tensor_mul(tmp[:], tmp[:], tmp[:])
            nc.vector.tensor_scalar(var[:], var[:], 1.0 / 128.0, None, ALU.mult)
            nc.vector.tensor_scalar_mul(tmp[:], tmp[:], 0.25)
            nc.vector.tensor_add(var[:], var[:], tmp[:])
            # sc = rsqrt(var+eps) = exp(-0.5*ln(var+eps)); nbias = -mean*sc
            sc = msc.tile([128, 1], DT.float32, tag="msc")
            nc.vector.tensor_scalar_add(sc[:], var[:], EPS)
            nc.scalar.activation(sc[:], sc[:], AF.Ln)
            nc.scalar.activation(sc[:], sc[:], AF.Exp, scale=-0.5)
            nbias = msc.tile([128, 1], DT.float32, tag="mnb")
            nc.vector.tensor_mul(nbias[:], mean[:], sc[:])
            nc.vector.tensor_scalar_mul(nbias[:], nbias[:], -1.0)
            ktm = msc.tile([128, 128], DT.float32, tag="ktm")
            nc.scalar.activation(ktm[:], ps4[:], AF.Identity, bias=nbias[:], scale=sc[:])
            # bf16 masked row-major copies + transposed copy
            nc.scalar.copy(krt_eo[0:64, ro * 256:ro * 256 + 128], ktm[0:64, :])
            nc.scalar.copy(krt_eo[64:128, ro * 256 + 128:ro * 256 + 256], ktm[64:128, :])
            ptk = dn_ps.tile([128, 128], DT.float32, tag="dn")
            nc.tensor.transpose(ptk[:], ktm[:], ident_f32[:])
            nc.scalar.copy(k2t[:, bass.ts(ro, 128)], ptk[:])

    # ---------------- attention over token chunks ----------------
    q_pool = ctx.enter_context(tc.tile_pool(name="qin", bufs=10))
    qs_pool = ctx.enter_context(tc.tile_pool(name="qs", bufs=8))
    qsT_pool = ctx.enter_context(tc.tile_pool(name="qsT", bufs=3))
    e_pool = ctx.enter_context(tc.tile_pool(name="eexp", bufs=10))
    e2_pool = ctx.enter_context(tc.tile_pool(name="ehat", bufs=10))
    o_pool = ctx.enter_context(tc.tile_pool(name="osb", bufs=4))
    stat_pool = ctx.enter_context(tc.tile_pool(name="stat", bufs=10))
    tp_ps = ctx.enter_context(tc.tile_pool(name="tp_ps", bufs=2, space="PSUM"))
    lg_ps = ctx.enter_context(tc.tile_pool(name="lg_ps", bufs=2, space="PSUM"))
    op_ps = ctx.enter_context(tc.tile_pool(name="op_ps", bufs=2, space="PSUM"))
    dn_ps = ctx.enter_context(tc.tile_pool(name="dn_ps", bufs=2, space="PSUM"))

    qscale_pool = int(os.environ.get("QSCALE_POOL", "0"))
    copy_dve = int(os.environ.get("COPY_DVE", "2"))
    evcopy_dve = int(os.environ.get("EVCOPY_DVE", "0"))

    def front(c0):
        """DMA + stats + scale + transpose for one chunk -> qsT tile."""
        q_t = []
        qs_t = []
        for t in range(TPC):
            qt = q_pool.tile([128, D], DT.bfloat16, tag="qt")
            nc.sync.dma_start(qt[:], q_ext[c0 + t * 128:c0 + t * 128 + 128, :])
            q_t.append(qt)
        for t in range(TPC):
            qt = q_t[t]
            st = stat_pool.tile([128, 16, 6], DT.float32, tag="st")
            # HW BNStats requires exactly 6 outputs/partition -> one instr per block
            for m in range(NB):
                nc.vector.bn_stats(st[:, m, :], qt[:, bass.ts(m, 128)])
            # V = 128*var = (st2+st5) + 32*(st1-st4)^2
            var = stat_pool.tile([128, 16], DT.float32, tag="var")
            tmp = stat_pool.tile([128, 16], DT.float32, tag="tmp")
            nc.vector.tensor_sub(tmp[:], st[:, :, 1], st[:, :, 4])
            nc.vector.scalar_tensor_tensor(tmp[:], tmp[:], 32.0, tmp[:], ALU.mult, ALU.mult)
            nc.vector.tensor_add(var[:], st[:, :, 2], st[:, :, 5])
            nc.vector.tensor_add(var[:], var[:], tmp[:])
            # c = rsqrt((var+eps)*DB) = exp(-0.5*ln(V + DB*eps))
            ct = stat_pool.tile([128, 16], DT.float32, tag="ct")
            nc.scalar.activation(ct[:], var[:], AF.Ln, bias=epsb[:])
            nc.scalar.activation(ct[:], ct[:], AF.Exp, scale=-0.5)
            qs = qs_pool.tile([128, D], DT.bfloat16, tag="qst")
            for m in range(NB):
                if (m * qscale_pool) % 16 < qscale_pool:
                    nc.gpsimd.tensor_scalar_mul(
                        qs[:, bass.ts(m, 128)], qt[:, bass.ts(m, 128)], ct[:, m:m + 1])
                else:
                    nc.vector.tensor_scalar_mul(
                        qs[:, bass.ts(m, 128)], qt[:, bass.ts(m, 128)], ct[:, m:m + 1])
            qs_t.append(qs)

        # transpose scaled q to feature-major: qsT[:, m*CHUNK + t*128 ...]
        # tp holds TWO m-blocks -> one [128, 1024] copy per pair
        qsT = qsT_pool.tile([128, NB * CHUNK], DT.bfloat16, tag="qsT")
        for jp in range(NB // 2):
            tp = tp_ps.tile([128, 2 * CHUNK], DT.bfloat16, tag="tp")
            for half in range(2):
                m = 2 * jp + half
                for t in range(TPC):
                    nc.tensor.transpose(
                        tp[:, half * CHUNK + t * 128:half * CHUNK + t * 128 + 128],
                        qs_t[t][:, bass.ts(m, 128)], ident_bf[:])
            # split copy between scalar and vector engines
            if (jp * copy_dve) % 8 < copy_dve:
                nc.vector.tensor_copy(qsT[:, bass.ts(jp, 2 * CHUNK)], tp[:])
            else:
                nc.scalar.copy(qsT[:, bass.ts(jp, 2 * CHUNK)], tp[:])
        return qsT

    total_chunks = n_chunks * repeats
    PRE = min(2, total_chunks)
    pending = {}
    for i in range(PRE):
        pending[i] = front((i % n_chunks) * CHUNK)
    # MLP emitted after the first fronts: its PE/ACT work overlaps the
    # initial q DMA + stats + scale + transposes
    with ExitStack() as sctx:
        emit_mlp(sctx)

    for it in range(total_chunks):
        chunk = it % n_chunks
        c0 = chunk * CHUNK
        if it + PRE < total_chunks and it + PRE not in pending:
            pending[it + PRE] = front(((it + PRE) % n_chunks) * CHUNK)
        qsT = pending.pop(it)

        # paired logits + exp: pair j covers m=2j (psum rows 0:64), m=2j+1 (rows 64:128)
        e_j = []
        for j in range(NB // 2):
            lg = lg_ps.tile([128, CHUNK], DT.float32, tag="lg")
            nc.tensor.matmul(
                lg[0:64, :], k2t[:, j * 128:j * 128 + 64],
                qsT[:, bass.ts(2 * j, CHUNK)], start=True, stop=True)
            nc.tensor.matmul(
                lg[64:128, :], k2t[:, j * 128 + 64:j * 128 + 128],
                qsT[:, bass.ts(2 * j + 1, CHUNK)], start=True, stop=True,
                tile_position=(0, 64))
            ej = e_pool.tile([128, CHUNK], DT.bfloat16, tag="ej")
            nc.scalar.activation(ej[:], lg[:], AF.Exp)
            e_j.append(ej)

        # denominators (token-major) -> reciprocal -> transpose -> replicate
        rT_ps = lg_ps.tile([128, CHUNK], DT.float32, tag="lg")
        for t in range(TPC):
            dn = dn_ps.tile([128, 16], DT.float32, tag="dn")
            # denominators: one N=2 matmul per pair, full-K stationary
            for j in range(NB // 2):
                nc.tensor.matmul(
                    dn[:, 2 * j:2 * j + 2], e_j[j][:, bass.ts(t, 128)],
                    ones2_bf[:], start=True, stop=True)
            rct = stat_pool.tile([128, 16], DT.float32, tag="rct")
            nc.vector.reciprocal(rct[:], dn[:])
            nc.tensor.transpose(rT_ps[0:16, bass.ts(t, 128)], rct[:], ident_f32[:])
        rT_sb = o_pool.tile([16, CHUNK], DT.bfloat16, tag="rTsb")
        nc.vector.tensor_copy(rT_sb[:], rT_ps[0:16, :])

        # e-side normalize: ehat_j[p, tok] = e_j[p, tok] * r[m(p), tok].
        # rrep = S_j.T @ rT -- S_j is a 0/1 selection matrix replicating rT
        # rows (2j, 2j+1) onto the partition halves.  (A bf16 is_transpose
        # variant was tried for 2x DVE reads but computes garbage on HW.)
        eh_j = []
        for j in range(NB // 2):
            rrep = op_ps.tile([128, CHUNK], DT.float32, tag="op")
            nc.tensor.matmul(
                rrep[:], s_all[:, bass.ts(j, 128)], rT_sb[:],
                start=True, stop=True, tile_position=(0, 0))
            eh = e2_pool.tile([128, CHUNK], DT.bfloat16, tag="eh")
            nc.vector.tensor_mul(eh[:], e_j[j][:], rrep[:])
            eh_j.append(eh)

        # out-projection: normalized already -> plain evict copies (cast to bf16).
        # In the drain (last iterations) the fronts are done and DVE idles, so
        # ramp the evict split toward DVE there.
        if it >= total_chunks - 2:
            ev = max(evcopy_dve, 3 if it == total_chunks - 1 else 2)
        else:
            ev = evcopy_dve
        for t in range(TPC):
            osb = o_pool.tile([128, D], DT.bfloat16, tag="osb")
            for g in range(4):
                op = op_ps.tile([128, 512], DT.float32, tag="op")
                for ji in range(2):
                    j = 2 * g + ji
                    nc.tensor.matmul(
                        op[:, bass.ts(ji, 256)], eh_j[j][:, bass.ts(t, 128)],
                        krt_eo[:, bass.ts(j, 256)], start=True, stop=True)
                if (g * ev) % 4 < ev:
                    nc.vector.tensor_copy(osb[:, bass.ts(g, 512)], op[:])
                else:
                    nc.scalar.copy(osb[:, bass.ts(g, 512)], op[:])
            nc.gpsimd.dma_start(out_ext[c0 + t * 128:c0 + t * 128 + 128, :], osb[:])


# ---------------------------------------------------------------------------
# host-side wrapper
# ---------------------------------------------------------------------------

_BUILD_CACHE = {}


def _split_multi_waits(nc):
    """walrus here allows at most one semaphore wait per instruction; hoist
    extras onto preceding same-engine NOPs (engine blocks on them in order)."""
    n = 0
    for f in nc.m.functions:
        for blk in f.blocks:
            new = []
            for inst in blk.instructions:
                si = getattr(inst, "sync_info", None)
                if si is not None and si.on_wait and len(si.on_wait) > 1:
                    waits = list(si.on_wait)
                    for w in waits[:-1]:
                        n += 1
                        new.append(mybir.InstNoOp(
                            name=f"{inst.name}_w{n}",
                            ins=[], outs=[],
                            engine=inst.engine,
                            sync_info=mybir.SyncInfo(on_wait=[w], on_update=[]),
                            bass_nofuse=True,
                        ))
                    si.on_wait = [waits[-1]]
                new.append(inst)
            blk.instructions = new
    return n


def _build(n_tokens=N_TOKENS, repeats=1):
    key = (n_tokens, repeats)
    if key in _BUILD_CACHE:
        return _BUILD_CACHE[key]
    nc = bass.Bass("TRN2", target_bir_lowering=False, debug=False, num_devices=N_CORES)
    ins = {
        "q": nc.declare_dram_parameter("q", [n_tokens, D], DT.bfloat16, isOutput=False)[:],
        "mp": nc.declare_dram_parameter("mp", [DB, NB * P], DT.bfloat16, isOutput=False)[:],
        "w1": nc.declare_dram_parameter("w1", [DB, H], DT.bfloat16, isOutput=False)[:],
        "w2": nc.declare_dram_parameter("w2", [H, H], DT.bfloat16, isOutput=False)[:],
        "w3": nc.declare_dram_parameter("w3", [H, H], DT.bfloat16, isOutput=False)[:],
        "w4": nc.declare_dram_parameter("w4", [H, DB], DT.bfloat16, isOutput=False)[:],
        "b123": nc.declare_dram_parameter("b123", [128, 12], DT.float32, isOutput=False)[:],
        "b4r": nc.declare_dram_parameter("b4r", [1, 128], DT.bfloat16, isOutput=False)[:],
        "sall": nc.declare_dram_parameter("sall", [NB, NB // 2 * 128], DT.bfloat16, isOutput=False)[:],
    }
    outs = {
        "out": nc.declare_dram_parameter("out", [n_tokens, D], DT.bfloat16, isOutput=True)[:],
    }
    with ExitStack() as ctx:
        tc = ctx.enter_context(tile.TileContext(nc))
        emit_kernel(ctx, tc, outs, ins, n_tokens=n_tokens, repeats=repeats)
    _split_multi_waits(nc)
    _BUILD_CACHE[key] = nc
    return nc


def _host_prep(queries, mem_params, W1, b1, W2, b2, W3, b3, W4, b4):
    q_bf = np.asarray(queries).astype(ml_dtypes.bfloat16)
    bf = lambda a: np.ascontiguousarray(np.asarray(a, dtype=np.float32)).astype(ml_dtypes.bfloat16)
    # feature-major mem_params, cols ordered (m, p): col = m*64 + p
    mp = np.ascontiguousarray(
        np.asarray(mem_params).reshape(P, NB, DB).transpose(2, 1, 0).reshape(DB, NB * P))
    b123 = np.concatenate(
        [np.asarray(b).reshape(4, 128).T for b in (b1, b2, b3)], axis=1
    ).astype(np.float32)
    b123 = np.ascontiguousarray(b123)
    s_all = np.zeros((NB, NB // 2 * 128), np.float32)
    for j in range(NB // 2):
        s_all[2 * j, j * 128:j * 128 + 64] = 1.0
        s_all[2 * j + 1, j * 128 + 64:j * 128 + 128] = 1.0
    common = {
        "sall": s_all.astype(ml_dtypes.bfloat16),
        "mp": bf(mp),
        "w1": bf(W1),
        "w2": bf(W2),
        "w3": bf(W3),
        "w4": bf(W4),
        "b123": b123,
        "b4r": bf(np.asarray(b4).reshape(1, 128)),
    }
    in_maps = []
    for b in range(N_CORES):
        m = dict(common)
        m["q"] = np.ascontiguousarray(q_bf[b])
        in_maps.append(m)
    return in_maps


def kernel(queries, mem_params, W1, b1, W2, b2, W3, b3, W4, b4):
    nc = _build(N_TOKENS)
    in_maps = _host_prep(queries, mem_params, W1, b1, W2, b2, W3, b3, W4, b4)
    trace = bool(int(os.environ.get("KERNEL_TRACE", "0")))
    try:
        res = run_bass_kernel_spmd(nc, in_maps, list(range(N_CORES)), trace=trace)
    except ModuleNotFoundError:
        res = run_bass_kernel_spmd(nc, in_maps, list(range(N_CORES)), trace=False)
    kernel.last_exec_time_ns = res.exec_time_ns
    kernel.last_results = res
    out = np.stack([res.results[i]["out"] for i in range(N_CORES)], axis=0)
    return out.astype(np.float32)


kernel.last_exec_time_ns = None

